# revision 11
# baseline (speedup 1.0000x reference)
import sys

sys.path.insert(0, "/opt/trn_rl_repo")

import numpy as np
import ml_dtypes

from concourse import bass, mybir
from concourse.tile import TileContext
from concourse.bass_utils import run_bass_kernel_spmd

dt = mybir.dt
Alu = mybir.AluOpType
Act = mybir.ActivationFunctionType

H = 4096
W = 4096
NCORES = 8
RPC = H // NCORES            # 512 output rows per core
HALO = 4                     # blur(2) + sobel(1) + nms(1)
SH = RPC + 2 * HALO          # 520 input rows per core
BASES = (0, 120, 240, 360, 392)
NT = len(BASES)
NCH = 3
P = 128
XW = W + 8                   # x layout: image col j at sbuf col j+4
BW = W + 4                   # blur/grad layout: image col j at sbuf col j+2
GW = W + 12                  # gU/gD layout: image col j at sbuf col j+3 (odd)
QW = 1024                    # quarter width (2 psum banks)
NQ = W // QW
HW_ = W // 2                 # NMS half width
BF16 = ml_dtypes.bfloat16

TAN_LO2 = float(np.float32(np.tan(3.14159 / 8)) ** 2)
TAN_HI2 = float(np.float32(np.tan(3 * 3.14159 / 8)) ** 2)
LOWER_T = 6.0
UPPER_T = 50.0

GO_T0 = 5 * P
GO_MID = GO_T0 + 4 * P
GO_T4 = GO_MID + 4 * P
WBW = GO_T4 + 4 * P          # 17*128 = 2176


def _band(taps, r):
    L = np.zeros((P, P), np.float32)
    for i, tv in enumerate(taps):
        L += np.float32(tv) * np.eye(P, k=r - i, dtype=np.float32)
    return L


def _weights(gauss, is_top, is_bot):
    g = np.asarray(gauss, np.float32)
    bg = _band(g, 2)
    v121 = _band([1.0, 2.0, 1.0], 1)
    u = _band([1.0, 0.0, -1.0], 1)

    def group(zero_rc):
        mats = [v121.copy(), -v121.copy(), u.copy(), 2.0 * u]
        if zero_rc is not None:
            for m in mats:
                m[zero_rc, :] = 0.0   # treat blur row as 0 (sobel pad)
                m[:, zero_rc] = 0.0   # force grad row to 0 (nms pad)
        return mats

    cols = [bg * g[d] for d in range(5)]
    cols += group(3 if is_top else None)       # t0 variant
    cols += group(None)                        # mid
    cols += group(124 if is_bot else None)     # t4 variant
    wb = np.concatenate(cols, axis=1)
    assert wb.shape == (P, WBW)
    return wb.astype(BF16)


def _build_nc():
    nc = bass.Bass(trn_type="TRN2")
    x_d = nc.dram_tensor("x", (NCH, SH, XW), dt.bfloat16, kind="ExternalInput")
    wb_d = nc.dram_tensor("wb", (P, WBW), dt.bfloat16, kind="ExternalInput")
    out_d = nc.dram_tensor("out", (NT * P, W), dt.uint8, kind="ExternalOutput")

    with TileContext(nc) as tc:
        with tc.tile_pool(name="sb", bufs=2) as pool, \
             tc.tile_pool(name="ps", bufs=2, space="PSUM") as pp:
            wb_sb = pool.tile([P, WBW], dt.bfloat16, tag="wb", bufs=1)
            nc.sync.dma_start(wb_sb[:, :], wb_d[:, :])

            _pq_n = [0]

            def psq():
                _pq_n[0] += 1
                return pp.tile([P, QW], dt.float32, tag="pq", bufs=4,
                               name=f"pq{_pq_n[0]}")

            def issue_x(t):
                base = BASES[t]
                xs = []
                for c in range(NCH):
                    x_sb = pool.tile([P, XW], dt.bfloat16, tag="x", bufs=4,
                                     name=f"x{t}_{c}")
                    nc.sync.dma_start(x_sb[:, :], x_d[c, base:base + P, :])
                    xs.append(x_sb)
                return xs

            def phase1a(t, xs):
                """blur for tile t."""
                blurs = []
                for c in range(NCH):
                    bl = pool.tile([P, BW], dt.bfloat16, tag=f"bl{c}", bufs=2)
                    nc.vector.memset(bl[:, 0:2], 0.0)
                    nc.vector.memset(bl[:, BW - 2:BW], 0.0)
                    for q in range(NQ):
                        ps = psq()
                        for d in range(5):
                            for ch in range(2):
                                j0 = q * QW + ch * 512
                                nc.tensor.matmul(
                                    out=ps[:, ch * 512:(ch + 1) * 512],
                                    lhsT=wb_sb[:, d * P:(d + 1) * P],
                                    rhs=xs[c][:, j0 + 2 + d:j0 + 2 + d + 512],
                                    start=(d == 0), stop=(d == 4),
                                )
                        if q % 2 == 0:
                            nc.scalar.activation(
                                bl[:, 2 + q * QW:2 + (q + 1) * QW],
                                ps[:, :], Act.Copy)
                        else:
                            nc.vector.tensor_scalar(
                                bl[:, 2 + q * QW:2 + (q + 1) * QW],
                                ps[:, :], 1.0, None, Alu.mult)
                    blurs.append(bl)
                return blurs

            def phase1b(t, blurs):
                """sobel + magnitude + orientation masks for tile t."""
                go = GO_T0 if t == 0 else (GO_T4 if t == NT - 1 else GO_MID)

                # blS = bl0 + bl1 + bl2 via gpsimd dma accumulate
                blS = pool.tile([P, BW], dt.bfloat16, tag="blS", bufs=1)
                nc.vector.tensor_tensor(blS[:, :], blurs[0][:, :], blurs[1][:, :],
                                        Alu.add)
                nc.vector.tensor_tensor(blS[:, :], blS[:, :], blurs[2][:, :],
                                        Alu.add)

                grad = pool.tile([P, BW], dt.bfloat16, tag="grad", bufs=2)
                nc.vector.memset(grad[:, 0:2], 0.0)
                nc.vector.memset(grad[:, BW - 2:BW], 0.0)

                def sobel_mm(src, q):
                    pgx = psq()
                    pgy = psq()
                    for i, (col, off) in enumerate(((go, 1), (go + P, 3))):
                        for ch in range(2):
                            jc = q * QW + ch * 512
                            nc.tensor.matmul(
                                out=pgx[:, ch * 512:(ch + 1) * 512],
                                lhsT=wb_sb[:, col:col + P],
                                rhs=src[:, jc + off:jc + off + 512],
                                start=(i == 0), stop=(i == 1),
                            )
                    for i, (col, off) in enumerate(
                            ((go + 2 * P, 1), (go + 3 * P, 2), (go + 2 * P, 3))):
                        for ch in range(2):
                            jc = q * QW + ch * 512
                            nc.tensor.matmul(
                                out=pgy[:, ch * 512:(ch + 1) * 512],
                                lhsT=wb_sb[:, col:col + P],
                                rhs=src[:, jc + off:jc + off + 512],
                                start=(i == 0), stop=(i == 2),
                            )
                    return pgx, pgy

                for c in range(NCH):
                    for q in range(NQ):
                        pgx, pgy = sobel_mm(blurs[c], q)
                        sqx = pool.tile([P, QW], dt.bfloat16, tag="sqx", bufs=2)
                        nc.scalar.activation(sqx[:, :], pgx[:, :], Act.Square)
                        sqy = pool.tile([P, QW], dt.bfloat16, tag="sqy", bufs=2)
                        nc.scalar.activation(sqy[:, :], pgy[:, :], Act.Square)
                        ss = pool.tile([P, QW], dt.bfloat16, tag="ss", bufs=2)
                        nc.vector.tensor_tensor(ss[:, :], sqx[:, :], sqy[:, :],
                                                Alu.add)
                        gslice = grad[:, 2 + q * QW:2 + (q + 1) * QW]
                        if c == 0:
                            nc.scalar.activation(gslice, ss[:, :], Act.Sqrt)
                        else:
                            mg = pool.tile([P, QW], dt.bfloat16, tag="mg", bufs=2)
                            nc.scalar.activation(mg[:, :], ss[:, :], Act.Sqrt)
                            nc.vector.tensor_tensor(gslice, gslice, mg[:, :],
                                                    Alu.add)

                csM = pool.tile([P, W], dt.uint8, tag="csM", bufs=2)
                c0M = pool.tile([P, W], dt.uint8, tag="c0M", bufs=2)
                c2M = pool.tile([P, W], dt.uint8, tag="c2M", bufs=2)
                for q in range(NQ):
                    pgx, pgy = sobel_mm(blS, q)
                    gxsb = pool.tile([P, QW], dt.bfloat16, tag="gxsb", bufs=2)
                    nc.scalar.activation(gxsb[:, :], pgx[:, :], Act.Copy)
                    gysb = pool.tile([P, QW], dt.bfloat16, tag="gysb", bufs=2)
                    nc.vector.tensor_scalar(gysb[:, :], pgy[:, :], 1.0, None,
                                            Alu.mult)
                    qs = slice(q * QW, (q + 1) * QW)
                    sxy = pool.tile([P, QW], dt.bfloat16, tag="sxy", bufs=1)
                    nc.vector.tensor_tensor(sxy[:, :], gxsb[:, :], gysb[:, :],
                                            Alu.mult)
                    nc.vector.tensor_scalar(csM[:, qs], sxy[:, :], 0.0, None,
                                            Alu.is_gt)
                    gx2 = pool.tile([P, QW], dt.bfloat16, tag="gx2", bufs=1)
                    nc.vector.tensor_tensor(gx2[:, :], gxsb[:, :], gxsb[:, :],
                                            Alu.mult)
                    gy2 = pool.tile([P, QW], dt.bfloat16, tag="gy2", bufs=1)
                    nc.vector.tensor_tensor(gy2[:, :], gysb[:, :], gysb[:, :],
                                            Alu.mult)
                    nc.vector.scalar_tensor_tensor(c2M[:, qs], gx2[:, :], TAN_HI2,
                                                   gy2[:, :], Alu.mult, Alu.is_lt)
                    nc.vector.scalar_tensor_tensor(c0M[:, qs], gx2[:, :], TAN_LO2,
                                                   gy2[:, :], Alu.mult, Alu.is_gt)

                # grad partition shifts: gU[m]=grad[m-1], gD[m]=grad[m+1]
                # (image col j at col j+3); issued here so transfers overlap
                gU = pool.tile([P, GW], dt.bfloat16, tag="gU", bufs=1)
                gD = pool.tile([P, GW], dt.bfloat16, tag="gD", bufs=1)
                hb = BW // 2
                for hh in range(2):
                    cs_ = slice(hh * hb, (hh + 1) * hb)
                    cd_ = slice(1 + hh * hb, 1 + (hh + 1) * hb)
                    nc.sync.dma_start(gU[1:P, cd_], grad[0:P - 1, cs_])
                    nc.sync.dma_start(gD[0:P - 1, cd_], grad[1:P, cs_])
                return {"grad": grad, "csM": csM, "c0M": c0M, "c2M": c2M,
                        "gU": gU, "gD": gD}

            def phase2(t, st):
                """NMS + threshold + output, tile t."""
                grad = st["grad"]
                gU, gD = st["gU"], st["gD"]

                ob = pool.tile([P, W], dt.uint8, tag="ob", bufs=1)
                for h in range(2):
                    hs = slice(h * HW_, (h + 1) * HW_)
                    g0 = 2 + h * HW_
                    u0 = 3 + h * HW_
                    m1 = pool.tile([P, HW_], dt.bfloat16, tag="m1", bufs=1)
                    nc.vector.tensor_tensor(m1[:, :], gD[:, u0 + 1:u0 + 1 + HW_],
                                            gU[:, u0 - 1:u0 - 1 + HW_], Alu.max)
                    msel = pool.tile([P, HW_], dt.bfloat16, tag="msel", bufs=1)
                    nc.vector.tensor_tensor(msel[:, :], gD[:, u0 - 1:u0 - 1 + HW_],
                                            gU[:, u0 + 1:u0 + 1 + HW_], Alu.max)
                    m0 = pool.tile([P, HW_], dt.bfloat16, tag="m0", bufs=1)
                    nc.vector.tensor_tensor(m0[:, :], grad[:, g0 - 1:g0 - 1 + HW_],
                                            grad[:, g0 + 1:g0 + 1 + HW_], Alu.max)
                    m2u = pool.tile([P, HW_], dt.bfloat16, tag="m2u", bufs=1)
                    nc.vector.tensor_tensor(m2u[:, :], gU[:, u0:u0 + HW_],
                                            gD[:, u0:u0 + HW_], Alu.max)
                    nc.vector.copy_predicated(msel[:, :], st["csM"][:, hs], m1[:, :])
                    nc.vector.copy_predicated(msel[:, :], st["c0M"][:, hs], m0[:, :])
                    nc.vector.copy_predicated(msel[:, :], st["c2M"][:, hs], m2u[:, :])
                    ig = pool.tile([P, HW_], dt.bfloat16, tag="ig", bufs=1)
                    nc.vector.scalar_tensor_tensor(ig[:, :], msel[:, :], LOWER_T,
                                                   grad[:, g0:g0 + HW_],
                                                   Alu.max, Alu.is_lt)
                    nc.vector.scalar_tensor_tensor(ob[:, hs], grad[:, g0:g0 + HW_],
                                                   UPPER_T, ig[:, :],
                                                   Alu.is_le, Alu.mult)
                nc.sync.dma_start(out_d[t * P:(t + 1) * P, :], ob[:, :])

            # pipeline with x prefetch: blur(t) -> P2(t-1) -> P1b(t)
            xs_cur = issue_x(0)
            blurs = phase1a(0, xs_cur)
            xs_next = issue_x(1)
            st_prev = phase1b(0, blurs)
            for t in range(1, NT):
                blurs = phase1a(t, xs_next)
                if t + 1 < NT:
                    xs_next = issue_x(t + 1)
                phase2(t - 1, st_prev)
                st_prev = phase1b(t, blurs)
            phase2(NT - 1, st_prev)

    import bass_rust
    bass_rust.move_matmul_waits_to_ldweights(nc.m)
    bass_rust.generate_event_semaphores(nc)
    nc.finalize()
    return nc


def _shard_inputs(img, gauss):
    imgf = np.ascontiguousarray(img[0])  # [3, H, W] f32
    in_maps = []
    for k in range(NCORES):
        xk = np.zeros((NCH, SH, XW), dtype=BF16)
        lo = k * RPC - HALO
        hi = k * RPC + RPC + HALO
        s0, s1 = max(lo, 0), min(hi, H)
        xk[:, s0 - lo:s1 - lo, 4:W + 4] = imgf[:, s0:s1, :].astype(BF16)
        wbk = _weights(gauss, is_top=(k == 0), is_bot=(k == NCORES - 1))
        in_maps.append({"x": xk, "wb": wbk})
    return in_maps


def _assemble(results):
    full = np.zeros((H, W), dtype=np.float32)
    for k in range(NCORES):
        ok = np.asarray(results[k]["out"])
        r0 = k * RPC
        for t in range(NT - 1):
            full[r0 + 120 * t:r0 + 120 * t + 120] = ok[P * t + 4:P * t + 124]
        full[r0 + 480:r0 + 512] = ok[(NT - 1) * P + 92:(NT - 1) * P + 124]
    return full.reshape(1, 1, H, W)


def _run(img, gauss, trace=False):
    nc = _build_nc()
    in_maps = _shard_inputs(np.asarray(img, np.float32), np.asarray(gauss, np.float32))
    res = run_bass_kernel_spmd(nc, in_maps, core_ids=list(range(NCORES)), trace=trace)
    return _assemble(res.results), res.exec_time_ns


def kernel(img=None, gauss=None, sobel=None, dir_w=None, **_):
    out, _ns = _run(img, gauss)
    return out


# revision 12
# speedup vs baseline: 1.1092x; 1.1092x over previous
import sys

sys.path.insert(0, "/opt/trn_rl_repo")

import numpy as np
import ml_dtypes

from concourse import bass, mybir
from concourse.tile import TileContext
from concourse.bass_utils import run_bass_kernel_spmd

dt = mybir.dt
Alu = mybir.AluOpType
Act = mybir.ActivationFunctionType

H = 4096
W = 4096
NCORES = 8
RPC = H // NCORES            # 512 output rows per core
HALO = 4                     # blur(2) + sobel(1) + nms(1)
SH = RPC + 2 * HALO          # 520 input rows per core
BASES = (0, 120, 240, 360, 392)
NT = len(BASES)
NCH = 3
P = 128
XW = W + 8                   # x layout: image col j at sbuf col j+4
BW = W + 4                   # blur/grad layout: image col j at sbuf col j+2
GW = W + 12                  # gU/gD layout: image col j at sbuf col j+3 (odd)
QW = 1024                    # quarter width (2 psum banks)
NQ = W // QW
HW_ = W // 2                 # NMS half width
BF16 = ml_dtypes.bfloat16

TAN_LO2 = float(np.float32(np.tan(3.14159 / 8)) ** 2)
TAN_HI2 = float(np.float32(np.tan(3 * 3.14159 / 8)) ** 2)
LOWER_T = 6.0
UPPER_T = 50.0

GO_T0 = 5 * P
GO_MID = GO_T0 + 4 * P
GO_T4 = GO_MID + 4 * P
WBW = GO_T4 + 4 * P          # 17*128 = 2176


def _band(taps, r):
    L = np.zeros((P, P), np.float32)
    for i, tv in enumerate(taps):
        L += np.float32(tv) * np.eye(P, k=r - i, dtype=np.float32)
    return L


def _weights(gauss, is_top, is_bot):
    g = np.asarray(gauss, np.float32)
    bg = _band(g, 2)
    v121 = _band([1.0, 2.0, 1.0], 1)
    u = _band([1.0, 0.0, -1.0], 1)

    def group(zero_rc):
        mats = [v121.copy(), -v121.copy(), u.copy(), 2.0 * u]
        if zero_rc is not None:
            for m in mats:
                m[zero_rc, :] = 0.0   # treat blur row as 0 (sobel pad)
                m[:, zero_rc] = 0.0   # force grad row to 0 (nms pad)
        return mats

    cols = [bg * g[d] for d in range(5)]
    cols += group(3 if is_top else None)       # t0 variant
    cols += group(None)                        # mid
    cols += group(124 if is_bot else None)     # t4 variant
    wb = np.concatenate(cols, axis=1)
    assert wb.shape == (P, WBW)
    return wb.astype(BF16)


def _build_nc():
    nc = bass.Bass(trn_type="TRN2")
    x_d = nc.dram_tensor("x", (NCH, SH, XW), dt.bfloat16, kind="ExternalInput")
    wb_d = nc.dram_tensor("wb", (P, WBW), dt.bfloat16, kind="ExternalInput")
    out_d = nc.dram_tensor("out", (NT * P, W), dt.uint8, kind="ExternalOutput")

    with TileContext(nc) as tc:
        with tc.tile_pool(name="sb", bufs=2) as pool, \
             tc.tile_pool(name="ps", bufs=2, space="PSUM") as pp:
            wb_sb = pool.tile([P, WBW], dt.bfloat16, tag="wb", bufs=1)
            nc.sync.dma_start(wb_sb[:, :], wb_d[:, :])

            _pq_n = [0]

            def psq():
                _pq_n[0] += 1
                return pp.tile([P, QW], dt.float32, tag="pq", bufs=4,
                               name=f"pq{_pq_n[0]}")

            def issue_x(t):
                base = BASES[t]
                xs = []
                for c in range(NCH):
                    x_sb = pool.tile([P, XW], dt.bfloat16, tag="x", bufs=4,
                                     name=f"x{t}_{c}")
                    nc.sync.dma_start(x_sb[:, :], x_d[c, base:base + P, :])
                    xs.append(x_sb)
                return xs

            def phase1a(t, xs):
                """blur for tile t."""
                blurs = []
                for c in range(NCH):
                    bl = pool.tile([P, BW], dt.bfloat16, tag=f"bl{c}", bufs=2)
                    nc.vector.memset(bl[:, 0:2], 0.0)
                    nc.vector.memset(bl[:, BW - 2:BW], 0.0)
                    for q in range(NQ):
                        ps = psq()
                        for d in range(5):
                            for ch in range(2):
                                j0 = q * QW + ch * 512
                                nc.tensor.matmul(
                                    out=ps[:, ch * 512:(ch + 1) * 512],
                                    lhsT=wb_sb[:, d * P:(d + 1) * P],
                                    rhs=xs[c][:, j0 + 2 + d:j0 + 2 + d + 512],
                                    start=(d == 0), stop=(d == 4),
                                )
                        nc.scalar.activation(bl[:, 2 + q * QW:2 + (q + 1) * QW],
                                             ps[:, :], Act.Copy)
                    blurs.append(bl)
                return blurs

            def phase1b(t, blurs):
                """sobel + magnitude + orientation masks for tile t."""
                go = GO_T0 if t == 0 else (GO_T4 if t == NT - 1 else GO_MID)

                # blS = bl0 + bl1 + bl2 via gpsimd dma accumulate
                blS = pool.tile([P, BW], dt.bfloat16, tag="blS", bufs=1)
                nc.vector.tensor_tensor(blS[:, :], blurs[0][:, :], blurs[1][:, :],
                                        Alu.add)
                nc.vector.tensor_tensor(blS[:, :], blS[:, :], blurs[2][:, :],
                                        Alu.add)

                grad = pool.tile([P, BW], dt.bfloat16, tag="grad", bufs=2)
                nc.vector.memset(grad[:, 0:2], 0.0)
                nc.vector.memset(grad[:, BW - 2:BW], 0.0)
                gU = pool.tile([P, GW], dt.bfloat16, tag="gU", bufs=1)
                gD = pool.tile([P, GW], dt.bfloat16, tag="gD", bufs=1)

                def sobel_mm(src, q):
                    pgx = psq()
                    pgy = psq()
                    for i, (col, off) in enumerate(((go, 1), (go + P, 3))):
                        for ch in range(2):
                            jc = q * QW + ch * 512
                            nc.tensor.matmul(
                                out=pgx[:, ch * 512:(ch + 1) * 512],
                                lhsT=wb_sb[:, col:col + P],
                                rhs=src[:, jc + off:jc + off + 512],
                                start=(i == 0), stop=(i == 1),
                            )
                    for i, (col, off) in enumerate(
                            ((go + 2 * P, 1), (go + 3 * P, 2), (go + 2 * P, 3))):
                        for ch in range(2):
                            jc = q * QW + ch * 512
                            nc.tensor.matmul(
                                out=pgy[:, ch * 512:(ch + 1) * 512],
                                lhsT=wb_sb[:, col:col + P],
                                rhs=src[:, jc + off:jc + off + 512],
                                start=(i == 0), stop=(i == 2),
                            )
                    return pgx, pgy

                for c in range(NCH):
                    for q in range(NQ):
                        pgx, pgy = sobel_mm(blurs[c], q)
                        sqx = pool.tile([P, QW], dt.bfloat16, tag="sqx", bufs=2)
                        nc.scalar.activation(sqx[:, :], pgx[:, :], Act.Square)
                        sqy = pool.tile([P, QW], dt.bfloat16, tag="sqy", bufs=2)
                        nc.scalar.activation(sqy[:, :], pgy[:, :], Act.Square)
                        ss = pool.tile([P, QW], dt.bfloat16, tag="ss", bufs=2)
                        nc.vector.tensor_tensor(ss[:, :], sqx[:, :], sqy[:, :],
                                                Alu.add)
                        gslice = grad[:, 2 + q * QW:2 + (q + 1) * QW]
                        if c == 0:
                            nc.scalar.activation(gslice, ss[:, :], Act.Sqrt)
                        else:
                            mg = pool.tile([P, QW], dt.bfloat16, tag="mg", bufs=2)
                            nc.scalar.activation(mg[:, :], ss[:, :], Act.Sqrt)
                            nc.vector.tensor_tensor(gslice, gslice, mg[:, :],
                                                    Alu.add)
                        if c == NCH - 1:
                            # grad quarter final: issue partition-shift DMAs
                            lo = q * QW if q else 0
                            hi = (q + 1) * QW if q < NQ - 1 else BW
                            lo2 = 2 + q * QW if q else 0
                            hi2 = 2 + (q + 1) * QW if q < NQ - 1 else BW
                            cs_ = slice(lo2, hi2)
                            cd_ = slice(lo2 + 1, hi2 + 1)
                            nc.sync.dma_start(gU[1:P, cd_], grad[0:P - 1, cs_])
                            nc.sync.dma_start(gD[0:P - 1, cd_], grad[1:P, cs_])

                csM = pool.tile([P, W], dt.uint8, tag="csM", bufs=2)
                c0M = pool.tile([P, W], dt.uint8, tag="c0M", bufs=2)
                c2M = pool.tile([P, W], dt.uint8, tag="c2M", bufs=2)
                for q in range(NQ):
                    pgx, pgy = sobel_mm(blS, q)
                    gxsb = pool.tile([P, QW], dt.bfloat16, tag="gxsb", bufs=2)
                    nc.scalar.activation(gxsb[:, :], pgx[:, :], Act.Copy)
                    gysb = pool.tile([P, QW], dt.bfloat16, tag="gysb", bufs=2)
                    nc.scalar.activation(gysb[:, :], pgy[:, :], Act.Copy)
                    qs = slice(q * QW, (q + 1) * QW)
                    sxy = pool.tile([P, QW], dt.bfloat16, tag="sxy", bufs=1)
                    nc.vector.tensor_tensor(sxy[:, :], gxsb[:, :], gysb[:, :],
                                            Alu.mult)
                    nc.vector.tensor_scalar(csM[:, qs], sxy[:, :], 0.0, None,
                                            Alu.is_gt)
                    gx2 = pool.tile([P, QW], dt.bfloat16, tag="gx2", bufs=1)
                    nc.vector.tensor_tensor(gx2[:, :], gxsb[:, :], gxsb[:, :],
                                            Alu.mult)
                    gy2 = pool.tile([P, QW], dt.bfloat16, tag="gy2", bufs=1)
                    nc.vector.tensor_tensor(gy2[:, :], gysb[:, :], gysb[:, :],
                                            Alu.mult)
                    nc.vector.scalar_tensor_tensor(c2M[:, qs], gx2[:, :], TAN_HI2,
                                                   gy2[:, :], Alu.mult, Alu.is_lt)
                    nc.vector.scalar_tensor_tensor(c0M[:, qs], gx2[:, :], TAN_LO2,
                                                   gy2[:, :], Alu.mult, Alu.is_gt)

                return {"grad": grad, "csM": csM, "c0M": c0M, "c2M": c2M,
                        "gU": gU, "gD": gD}

            def phase2(t, st):
                """NMS + threshold + output, tile t."""
                grad = st["grad"]
                gU, gD = st["gU"], st["gD"]

                ob = pool.tile([P, W], dt.uint8, tag="ob", bufs=1)
                for h in range(2):
                    hs = slice(h * HW_, (h + 1) * HW_)
                    g0 = 2 + h * HW_
                    u0 = 3 + h * HW_
                    m1 = pool.tile([P, HW_], dt.bfloat16, tag="m1", bufs=1)
                    nc.vector.tensor_tensor(m1[:, :], gD[:, u0 + 1:u0 + 1 + HW_],
                                            gU[:, u0 - 1:u0 - 1 + HW_], Alu.max)
                    msel = pool.tile([P, HW_], dt.bfloat16, tag="msel", bufs=1)
                    nc.vector.tensor_tensor(msel[:, :], gD[:, u0 - 1:u0 - 1 + HW_],
                                            gU[:, u0 + 1:u0 + 1 + HW_], Alu.max)
                    m0 = pool.tile([P, HW_], dt.bfloat16, tag="m0", bufs=1)
                    nc.vector.tensor_tensor(m0[:, :], grad[:, g0 - 1:g0 - 1 + HW_],
                                            grad[:, g0 + 1:g0 + 1 + HW_], Alu.max)
                    m2u = pool.tile([P, HW_], dt.bfloat16, tag="m2u", bufs=1)
                    nc.vector.tensor_tensor(m2u[:, :], gU[:, u0:u0 + HW_],
                                            gD[:, u0:u0 + HW_], Alu.max)
                    nc.vector.copy_predicated(msel[:, :], st["csM"][:, hs], m1[:, :])
                    nc.vector.copy_predicated(msel[:, :], st["c0M"][:, hs], m0[:, :])
                    nc.vector.copy_predicated(msel[:, :], st["c2M"][:, hs], m2u[:, :])
                    ig = pool.tile([P, HW_], dt.bfloat16, tag="ig", bufs=1)
                    nc.vector.scalar_tensor_tensor(ig[:, :], msel[:, :], LOWER_T,
                                                   grad[:, g0:g0 + HW_],
                                                   Alu.max, Alu.is_lt)
                    nc.vector.scalar_tensor_tensor(ob[:, hs], grad[:, g0:g0 + HW_],
                                                   UPPER_T, ig[:, :],
                                                   Alu.is_le, Alu.mult)
                nc.sync.dma_start(out_d[t * P:(t + 1) * P, :], ob[:, :])

            # pipeline with x prefetch: blur(t) -> P2(t-1) -> P1b(t)
            xs_cur = issue_x(0)
            blurs = phase1a(0, xs_cur)
            xs_next = issue_x(1)
            st_prev = phase1b(0, blurs)
            for t in range(1, NT):
                blurs = phase1a(t, xs_next)
                if t + 1 < NT:
                    xs_next = issue_x(t + 1)
                phase2(t - 1, st_prev)
                st_prev = phase1b(t, blurs)
            phase2(NT - 1, st_prev)

    import bass_rust
    bass_rust.move_matmul_waits_to_ldweights(nc.m)
    bass_rust.generate_event_semaphores(nc)
    nc.finalize()
    return nc


def _shard_inputs(img, gauss):
    imgf = np.ascontiguousarray(img[0])  # [3, H, W] f32
    in_maps = []
    for k in range(NCORES):
        xk = np.zeros((NCH, SH, XW), dtype=BF16)
        lo = k * RPC - HALO
        hi = k * RPC + RPC + HALO
        s0, s1 = max(lo, 0), min(hi, H)
        xk[:, s0 - lo:s1 - lo, 4:W + 4] = imgf[:, s0:s1, :].astype(BF16)
        wbk = _weights(gauss, is_top=(k == 0), is_bot=(k == NCORES - 1))
        in_maps.append({"x": xk, "wb": wbk})
    return in_maps


def _assemble(results):
    full = np.zeros((H, W), dtype=np.float32)
    for k in range(NCORES):
        ok = np.asarray(results[k]["out"])
        r0 = k * RPC
        for t in range(NT - 1):
            full[r0 + 120 * t:r0 + 120 * t + 120] = ok[P * t + 4:P * t + 124]
        full[r0 + 480:r0 + 512] = ok[(NT - 1) * P + 92:(NT - 1) * P + 124]
    return full.reshape(1, 1, H, W)


def _run(img, gauss, trace=False):
    nc = _build_nc()
    in_maps = _shard_inputs(np.asarray(img, np.float32), np.asarray(gauss, np.float32))
    res = run_bass_kernel_spmd(nc, in_maps, core_ids=list(range(NCORES)), trace=trace)
    return _assemble(res.results), res.exec_time_ns


def kernel(img=None, gauss=None, sobel=None, dir_w=None, **_):
    out, _ns = _run(img, gauss)
    return out


# revision 15
# speedup vs baseline: 1.8510x; 1.6688x over previous
import sys

sys.path.insert(0, "/opt/trn_rl_repo")

import numpy as np
import ml_dtypes

from concourse import bass, mybir
from concourse.tile import TileContext
from concourse.bass_utils import run_bass_kernel_spmd

dt = mybir.dt
Alu = mybir.AluOpType
Act = mybir.ActivationFunctionType

H = 4096
W = 4096
NCORES = 8
RPC = H // NCORES            # 512 output rows per core
HALO = 4                     # blur(2) + sobel(1) + nms(1)
SH = RPC + 2 * HALO          # 520 input rows per core
BASES = (0, 120, 240, 360, 392)
NT = len(BASES)
NCH = 3
P = 128
QW = 1024                    # quarter width (2 psum banks)
NQ = W // QW
BF16 = ml_dtypes.bfloat16

TAN_LO2 = float(np.float32(np.tan(3.14159 / 8)) ** 2)
TAN_HI2 = float(np.float32(np.tan(3 * 3.14159 / 8)) ** 2)
LOWER_T = 6.0
UPPER_T = 50.0

# wb column layout: 5 blur bands then [V121, NV121, U, U2, SU, SD] x {mid, t0, t4}
GO_MID = 5 * P
GO_T0 = GO_MID + 6 * P
GO_T4 = GO_T0 + 6 * P
WBW = GO_T4 + 6 * P          # 2944


def _band(taps, r):
    L = np.zeros((P, P), np.float32)
    for i, tv in enumerate(taps):
        L += np.float32(tv) * np.eye(P, k=r - i, dtype=np.float32)
    return L


def _weights(gauss, is_top, is_bot):
    g = np.asarray(gauss, np.float32)
    bg = _band(g, 2)
    v121 = _band([1.0, 2.0, 1.0], 1)
    u = _band([1.0, 0.0, -1.0], 1)
    su = _band([1.0], 1)
    sd = _band([1.0], -1)

    def group(zero_row, zero_su, zero_sd):
        mats = [v121.copy(), -v121, u.copy(), 2.0 * u, su.copy(), sd.copy()]
        if zero_row is not None:
            for idx in (0, 1, 2, 3):
                mats[idx][zero_row, :] = 0.0
            if zero_su:
                mats[4][zero_row, :] = 0.0
            if zero_sd:
                mats[5][zero_row, :] = 0.0
        return mats

    cols = [bg * g[d] for d in range(5)]
    cols += group(None, False, False)                       # mid
    cols += group(3 if is_top else None, True, False)       # t0 variant
    cols += group(124 if is_bot else None, False, True)     # t4 variant
    wb = np.concatenate(cols, axis=1)
    assert wb.shape == (P, WBW)
    return wb.astype(BF16)


def _build_nc():
    nc = bass.Bass(trn_type="TRN2")
    x_d = nc.dram_tensor("x", (NCH, SH, W + 4), dt.bfloat16, kind="ExternalInput")
    wb_d = nc.dram_tensor("wb", (P, WBW), dt.bfloat16, kind="ExternalInput")
    out_d = nc.dram_tensor("out", (NT * P, W), dt.uint8, kind="ExternalOutput")

    with TileContext(nc) as tc:
        with tc.tile_pool(name="sb", bufs=2) as pool, \
             tc.tile_pool(name="ps", bufs=2, space="PSUM") as pp:
            wb_sb = pool.tile([P, WBW], dt.bfloat16, tag="wb", bufs=1)
            nc.sync.dma_start(wb_sb[:, :], wb_d[:, :])

            _pq_n = [0]

            def psq():
                _pq_n[0] += 1
                return pp.tile([P, QW], dt.float32, tag="pq", bufs=4,
                               name=f"pq{_pq_n[0]}")

            def issue_x(t):
                base = BASES[t]
                xs = []
                for c in range(NCH):
                    x_sb = pool.tile([P, W + 4], dt.bfloat16, tag="x", bufs=4,
                                     name=f"x{t}_{c}")
                    nc.sync.dma_start(x_sb[:, :], x_d[c, base:base + P, :])
                    xs.append(x_sb)
                return xs

            xs_cur = issue_x(0)
            for t in range(NT):
                base = BASES[t]
                go = GO_T0 if t == 0 else (GO_T4 if t == NT - 1 else GO_MID)

                # ---- stage A: fused separable 5x5 blur per channel ----
                blurs = []
                for c in range(NCH):
                    bl = pool.tile([P, W + 2], dt.bfloat16, tag=f"bl{c}", bufs=2)
                    nc.vector.memset(bl[:, 0:1], 0.0)
                    nc.vector.memset(bl[:, W + 1:W + 2], 0.0)
                    for q in range(NQ):
                        ps = psq()
                        for d in range(5):
                            for ch in range(2):
                                j0 = q * QW + ch * 512
                                nc.tensor.matmul(
                                    out=ps[:, ch * 512:(ch + 1) * 512],
                                    lhsT=wb_sb[:, d * P:(d + 1) * P],
                                    rhs=xs_cur[c][:, j0 + d:j0 + d + 512],
                                    start=(d == 0), stop=(d == 4),
                                )
                        nc.scalar.activation(bl[:, 1 + q * QW:1 + (q + 1) * QW],
                                             ps[:, :], Act.Copy)
                    blurs.append(bl)
                if t + 1 < NT:
                    xs_cur = issue_x(t + 1)

                # blS = bl0+bl1+bl2 over full buffers (even base -> 2x DVE)
                blS = pool.tile([P, W + 2], dt.bfloat16, tag="blS", bufs=1)
                nc.vector.tensor_tensor(blS[:, :], blurs[0][:, :],
                                        blurs[1][:, :], Alu.add)
                nc.vector.tensor_tensor(blS[:, :], blS[:, :],
                                        blurs[2][:, :], Alu.add)

                # ---- stage B: grad, masks, shifts (per quarter) ----
                grad = pool.tile([P, W + 2], dt.bfloat16, tag="grad", bufs=2)
                gU = pool.tile([P, W + 2], dt.bfloat16, tag="gU", bufs=1)
                gD = pool.tile([P, W + 2], dt.bfloat16, tag="gD", bufs=1)
                for bufv in (grad, gU, gD):
                    nc.vector.memset(bufv[:, 0:1], 0.0)
                    nc.vector.memset(bufv[:, W + 1:W + 2], 0.0)
                csM = pool.tile([P, W], dt.uint8, tag="csM", bufs=2)
                c0M = pool.tile([P, W], dt.uint8, tag="c0M", bufs=2)
                c2M = pool.tile([P, W], dt.uint8, tag="c2M", bufs=2)

                def sobel_mm(src, q):
                    pj = 1 + q * QW
                    pgx = psq()
                    pgy = psq()
                    for i, (col, off) in enumerate(((go, -1), (go + P, 1))):
                        for ch in range(2):
                            nc.tensor.matmul(
                                out=pgx[:, ch * 512:(ch + 1) * 512],
                                lhsT=wb_sb[:, col:col + P],
                                rhs=src[:, pj + off + ch * 512:
                                        pj + off + ch * 512 + 512],
                                start=(i == 0), stop=(i == 1),
                            )
                    for i, (col, off) in enumerate(
                            ((go + 2 * P, -1), (go + 3 * P, 0), (go + 2 * P, 1))):
                        for ch in range(2):
                            nc.tensor.matmul(
                                out=pgy[:, ch * 512:(ch + 1) * 512],
                                lhsT=wb_sb[:, col:col + P],
                                rhs=src[:, pj + off + ch * 512:
                                        pj + off + ch * 512 + 512],
                                start=(i == 0), stop=(i == 2),
                            )
                    return pgx, pgy

                for q in range(NQ):
                    pj = 1 + q * QW
                    qs = slice(q * QW, (q + 1) * QW)
                    for c in range(NCH):
                        pgx, pgy = sobel_mm(blurs[c], q)
                        sqx = pool.tile([P, QW], dt.bfloat16, tag="sqx", bufs=2)
                        nc.scalar.activation(sqx[:, :], pgx[:, :], Act.Square)
                        sqy = pool.tile([P, QW], dt.bfloat16, tag="sqy", bufs=2)
                        nc.scalar.activation(sqy[:, :], pgy[:, :], Act.Square)
                        ss = pool.tile([P, QW], dt.bfloat16, tag="ss", bufs=2)
                        nc.vector.tensor_tensor(ss[:, :], sqx[:, :], sqy[:, :],
                                                Alu.add)
                        gslice = grad[:, pj:pj + QW]
                        if c == 0:
                            nc.scalar.activation(gslice, ss[:, :], Act.Sqrt)
                        else:
                            mg = pool.tile([P, QW], dt.bfloat16, tag="mg", bufs=2)
                            nc.scalar.activation(mg[:, :], ss[:, :], Act.Sqrt)
                            nc.vector.tensor_tensor(gslice, gslice, mg[:, :],
                                                    Alu.add)
                    # orientation masks from summed-blur sobel
                    pgxs, pgys = sobel_mm(blS, q)
                    gxsb = pool.tile([P, QW], dt.bfloat16, tag="gxsb", bufs=2)
                    nc.scalar.activation(gxsb[:, :], pgxs[:, :], Act.Copy)
                    gysb = pool.tile([P, QW], dt.bfloat16, tag="gysb", bufs=2)
                    nc.scalar.activation(gysb[:, :], pgys[:, :], Act.Copy)
                    sxy = pool.tile([P, QW], dt.bfloat16, tag="sxy", bufs=2)
                    nc.vector.tensor_tensor(sxy[:, :], gxsb[:, :], gysb[:, :],
                                            Alu.mult)
                    nc.vector.tensor_scalar(csM[:, qs], sxy[:, :], 0.0, None,
                                            Alu.is_gt)
                    gx2 = pool.tile([P, QW], dt.bfloat16, tag="gx2", bufs=2)
                    nc.vector.tensor_tensor(gx2[:, :], gxsb[:, :], gxsb[:, :],
                                            Alu.mult)
                    gy2 = pool.tile([P, QW], dt.bfloat16, tag="gy2", bufs=2)
                    nc.vector.tensor_tensor(gy2[:, :], gysb[:, :], gysb[:, :],
                                            Alu.mult)
                    nc.vector.scalar_tensor_tensor(c2M[:, qs], gx2[:, :], TAN_HI2,
                                                   gy2[:, :], Alu.mult, Alu.is_lt)
                    nc.vector.scalar_tensor_tensor(c0M[:, qs], gx2[:, :], TAN_LO2,
                                                   gy2[:, :], Alu.mult, Alu.is_gt)

                # ---- stage C: shifts then NMS select + band (per quarter) ----
                ob = pool.tile([P, W], dt.uint8, tag="ob", bufs=1)
                for q in range(NQ):
                    pj = 1 + q * QW
                    # vertical shifts via SU/SD band matmuls (grad complete now)
                    pU = psq()
                    for ch in range(2):
                        nc.tensor.matmul(out=pU[:, ch * 512:(ch + 1) * 512],
                                         lhsT=wb_sb[:, go + 4 * P:go + 5 * P],
                                         rhs=grad[:, pj + ch * 512:
                                                  pj + ch * 512 + 512],
                                         start=True, stop=True)
                    nc.scalar.activation(gU[:, pj:pj + QW], pU[:, :], Act.Copy)
                    pD = psq()
                    for ch in range(2):
                        nc.tensor.matmul(out=pD[:, ch * 512:(ch + 1) * 512],
                                         lhsT=wb_sb[:, go + 5 * P:go + 6 * P],
                                         rhs=grad[:, pj + ch * 512:
                                                  pj + ch * 512 + 512],
                                         start=True, stop=True)
                    nc.vector.tensor_scalar(gD[:, pj:pj + QW], pD[:, :], 1.0,
                                            None, Alu.mult)
                for q in range(NQ):
                    pj = 1 + q * QW
                    qs = slice(q * QW, (q + 1) * QW)
                    m1 = pool.tile([P, QW], dt.bfloat16, tag="m1", bufs=1)
                    nc.vector.tensor_tensor(m1[:, :], gD[:, pj + 1:pj + 1 + QW],
                                            gU[:, pj - 1:pj - 1 + QW], Alu.max)
                    msel = pool.tile([P, QW], dt.bfloat16, tag="msel", bufs=1)
                    nc.vector.tensor_tensor(msel[:, :], gD[:, pj - 1:pj - 1 + QW],
                                            gU[:, pj + 1:pj + 1 + QW], Alu.max)
                    m0 = pool.tile([P, QW], dt.bfloat16, tag="m0", bufs=1)
                    nc.vector.tensor_tensor(m0[:, :], grad[:, pj - 1:pj - 1 + QW],
                                            grad[:, pj + 1:pj + 1 + QW], Alu.max)
                    m2u = pool.tile([P, QW], dt.bfloat16, tag="m2u", bufs=1)
                    nc.vector.tensor_tensor(m2u[:, :], gU[:, pj:pj + QW],
                                            gD[:, pj:pj + QW], Alu.max)
                    nc.vector.copy_predicated(msel[:, :], csM[:, qs], m1[:, :])
                    nc.vector.copy_predicated(msel[:, :], c0M[:, qs], m0[:, :])
                    nc.vector.copy_predicated(msel[:, :], c2M[:, qs], m2u[:, :])
                    ig = pool.tile([P, QW], dt.bfloat16, tag="ig", bufs=1)
                    nc.vector.scalar_tensor_tensor(ig[:, :], msel[:, :], LOWER_T,
                                                   grad[:, pj:pj + QW],
                                                   Alu.max, Alu.is_lt)
                    nc.vector.scalar_tensor_tensor(ob[:, qs], grad[:, pj:pj + QW],
                                                   UPPER_T, ig[:, :],
                                                   Alu.is_le, Alu.mult)
                nc.sync.dma_start(out_d[t * P:(t + 1) * P, :], ob[:, :])

    import bass_rust
    bass_rust.move_matmul_waits_to_ldweights(nc.m)
    bass_rust.generate_event_semaphores(nc)
    nc.finalize()
    return nc


def _shard_inputs(img, gauss):
    imgf = np.ascontiguousarray(img[0])  # [3, H, W] f32
    in_maps = []
    for k in range(NCORES):
        xk = np.zeros((NCH, SH, W + 4), dtype=BF16)
        lo = k * RPC - HALO
        hi = k * RPC + RPC + HALO
        s0, s1 = max(lo, 0), min(hi, H)
        xk[:, s0 - lo:s1 - lo, 2:W + 2] = imgf[:, s0:s1, :].astype(BF16)
        wbk = _weights(gauss, is_top=(k == 0), is_bot=(k == NCORES - 1))
        in_maps.append({"x": xk, "wb": wbk})
    return in_maps


def _assemble(results):
    full = np.zeros((H, W), dtype=np.float32)
    for k in range(NCORES):
        ok = np.asarray(results[k]["out"])
        r0 = k * RPC
        for t in range(NT - 1):
            full[r0 + 120 * t:r0 + 120 * t + 120] = ok[P * t + 4:P * t + 124]
        full[r0 + 480:r0 + 512] = ok[(NT - 1) * P + 92:(NT - 1) * P + 124]
    return full.reshape(1, 1, H, W)


def _run(img, gauss, trace=False):
    nc = _build_nc()
    in_maps = _shard_inputs(np.asarray(img, np.float32), np.asarray(gauss, np.float32))
    res = run_bass_kernel_spmd(nc, in_maps, core_ids=list(range(NCORES)), trace=trace)
    return _assemble(res.results), res.exec_time_ns


def kernel(img=None, gauss=None, sobel=None, dir_w=None, **_):
    out, _ns = _run(img, gauss)
    return out


# revision 16
# speedup vs baseline: 1.8933x; 1.0229x over previous
import sys

sys.path.insert(0, "/opt/trn_rl_repo")

import numpy as np
import ml_dtypes

from concourse import bass, mybir
from concourse.tile import TileContext
from concourse.bass_utils import run_bass_kernel_spmd

dt = mybir.dt
Alu = mybir.AluOpType
Act = mybir.ActivationFunctionType

H = 4096
W = 4096
NCORES = 8
RPC = H // NCORES            # 512 output rows per core
HALO = 4                     # blur(2) + sobel(1) + nms(1)
SH = RPC + 2 * HALO          # 520 input rows per core
BASES = (0, 120, 240, 360, 392)
NT = len(BASES)
NCH = 3
P = 128
QW = 1024                    # quarter width (2 psum banks)
NQ = W // QW
BF16 = ml_dtypes.bfloat16

TAN_LO2 = float(np.float32(np.tan(3.14159 / 8)) ** 2)
TAN_HI2 = float(np.float32(np.tan(3 * 3.14159 / 8)) ** 2)
LOWER_T = 6.0
UPPER_T = 50.0

# wb column layout: 5 blur bands then [V121, NV121, U, U2, SU, SD] x {mid, t0, t4}
GO_MID = 5 * P
GO_T0 = GO_MID + 6 * P
GO_T4 = GO_T0 + 6 * P
WBW = GO_T4 + 6 * P          # 2944


def _band(taps, r):
    L = np.zeros((P, P), np.float32)
    for i, tv in enumerate(taps):
        L += np.float32(tv) * np.eye(P, k=r - i, dtype=np.float32)
    return L


def _weights(gauss, is_top, is_bot):
    g = np.asarray(gauss, np.float32)
    bg = _band(g, 2)
    v121 = _band([1.0, 2.0, 1.0], 1)
    u = _band([1.0, 0.0, -1.0], 1)
    su = _band([1.0], 1)
    sd = _band([1.0], -1)

    def group(zero_row, zero_su, zero_sd):
        mats = [v121.copy(), -v121, u.copy(), 2.0 * u, su.copy(), sd.copy()]
        if zero_row is not None:
            for idx in (0, 1, 2, 3):
                mats[idx][zero_row, :] = 0.0
            if zero_su:
                mats[4][zero_row, :] = 0.0
            if zero_sd:
                mats[5][zero_row, :] = 0.0
        return mats

    cols = [bg * g[d] for d in range(5)]
    cols += group(None, False, False)                       # mid
    cols += group(3 if is_top else None, True, False)       # t0 variant
    cols += group(124 if is_bot else None, False, True)     # t4 variant
    wb = np.concatenate(cols, axis=1)
    assert wb.shape == (P, WBW)
    return wb.astype(BF16)


def _build_nc():
    nc = bass.Bass(trn_type="TRN2")
    x_d = nc.dram_tensor("x", (NCH, SH, W + 4), dt.bfloat16, kind="ExternalInput")
    wb_d = nc.dram_tensor("wb", (P, WBW), dt.bfloat16, kind="ExternalInput")
    out_d = nc.dram_tensor("out", (NT * P, W), dt.uint8, kind="ExternalOutput")

    with TileContext(nc) as tc:
        with tc.tile_pool(name="sb", bufs=2) as pool, \
             tc.tile_pool(name="ps", bufs=2, space="PSUM") as pp:
            wb_sb = pool.tile([P, WBW], dt.bfloat16, tag="wb", bufs=1)
            nc.sync.dma_start(wb_sb[:, :], wb_d[:, :])

            _pq_n = [0]

            def psq():
                _pq_n[0] += 1
                return pp.tile([P, QW], dt.float32, tag="pq", bufs=4,
                               name=f"pq{_pq_n[0]}")

            def issue_x(t):
                base = BASES[t]
                xs = []
                for c in range(NCH):
                    x_sb = pool.tile([P, W + 4], dt.bfloat16, tag="x", bufs=4,
                                     name=f"x{t}_{c}")
                    nc.sync.dma_start(x_sb[:, :], x_d[c, base:base + P, :])
                    xs.append(x_sb)
                return xs

            xs_cur = issue_x(0)
            for t in range(NT):
                base = BASES[t]
                go = GO_T0 if t == 0 else (GO_T4 if t == NT - 1 else GO_MID)

                # ---- stage A: fused separable 5x5 blur per channel ----
                blurs = []
                for c in range(NCH):
                    bl = pool.tile([P, W + 2], dt.bfloat16, tag=f"bl{c}", bufs=2)
                    nc.vector.memset(bl[:, 0:1], 0.0)
                    nc.vector.memset(bl[:, W + 1:W + 2], 0.0)
                    for q in range(NQ):
                        ps = psq()
                        for d in range(5):
                            for ch in range(2):
                                j0 = q * QW + ch * 512
                                nc.tensor.matmul(
                                    out=ps[:, ch * 512:(ch + 1) * 512],
                                    lhsT=wb_sb[:, d * P:(d + 1) * P],
                                    rhs=xs_cur[c][:, j0 + d:j0 + d + 512],
                                    start=(d == 0), stop=(d == 4),
                                )
                        nc.scalar.activation(bl[:, 1 + q * QW:1 + (q + 1) * QW],
                                             ps[:, :], Act.Copy)
                    blurs.append(bl)
                if t + 1 < NT:
                    xs_cur = issue_x(t + 1)

                # blS = bl0+bl1+bl2 over full buffers (even base -> 2x DVE)
                blS = pool.tile([P, W + 2], dt.bfloat16, tag="blS", bufs=1)
                nc.vector.tensor_tensor(blS[:, :], blurs[0][:, :],
                                        blurs[1][:, :], Alu.add)
                nc.vector.tensor_tensor(blS[:, :], blS[:, :],
                                        blurs[2][:, :], Alu.add)

                # ---- stage B: grad, masks, shifts (per quarter) ----
                grad = pool.tile([P, W + 2], dt.bfloat16, tag="grad", bufs=2)
                gU = pool.tile([P, W + 2], dt.bfloat16, tag="gU", bufs=1)
                gD = pool.tile([P, W + 2], dt.bfloat16, tag="gD", bufs=1)
                for bufv in (grad, gU, gD):
                    nc.vector.memset(bufv[:, 0:1], 0.0)
                    nc.vector.memset(bufv[:, W + 1:W + 2], 0.0)
                csM = pool.tile([P, W], dt.uint8, tag="csM", bufs=2)
                c0M = pool.tile([P, W], dt.uint8, tag="c0M", bufs=2)
                c2M = pool.tile([P, W], dt.uint8, tag="c2M", bufs=2)

                def sobel_mm(src, q):
                    pj = 1 + q * QW
                    pgx = psq()
                    pgy = psq()
                    for i, (col, off) in enumerate(((go, -1), (go + P, 1))):
                        for ch in range(2):
                            nc.tensor.matmul(
                                out=pgx[:, ch * 512:(ch + 1) * 512],
                                lhsT=wb_sb[:, col:col + P],
                                rhs=src[:, pj + off + ch * 512:
                                        pj + off + ch * 512 + 512],
                                start=(i == 0), stop=(i == 1),
                            )
                    for i, (col, off) in enumerate(
                            ((go + 2 * P, -1), (go + 3 * P, 0), (go + 2 * P, 1))):
                        for ch in range(2):
                            nc.tensor.matmul(
                                out=pgy[:, ch * 512:(ch + 1) * 512],
                                lhsT=wb_sb[:, col:col + P],
                                rhs=src[:, pj + off + ch * 512:
                                        pj + off + ch * 512 + 512],
                                start=(i == 0), stop=(i == 2),
                            )
                    return pgx, pgy

                for q in range(NQ):
                    pj = 1 + q * QW
                    qs = slice(q * QW, (q + 1) * QW)
                    for c in range(NCH):
                        pgx, pgy = sobel_mm(blurs[c], q)
                        sqx = pool.tile([P, QW], dt.bfloat16, tag="sqx", bufs=2)
                        nc.scalar.activation(sqx[:, :], pgx[:, :], Act.Square)
                        sqy = pool.tile([P, QW], dt.bfloat16, tag="sqy", bufs=2)
                        nc.scalar.activation(sqy[:, :], pgy[:, :], Act.Square)
                        ss = pool.tile([P, QW], dt.bfloat16, tag="ss", bufs=2)
                        nc.vector.tensor_tensor(ss[:, :], sqx[:, :], sqy[:, :],
                                                Alu.add)
                        gslice = grad[:, pj:pj + QW]
                        if c == 0:
                            nc.scalar.activation(gslice, ss[:, :], Act.Sqrt)
                        else:
                            mg = pool.tile([P, QW], dt.bfloat16, tag="mg", bufs=2)
                            nc.scalar.activation(mg[:, :], ss[:, :], Act.Sqrt)
                            nc.vector.tensor_tensor(gslice, gslice, mg[:, :],
                                                    Alu.add)
                    # orientation masks from summed-blur sobel
                    pgxs, pgys = sobel_mm(blS, q)
                    gxsb = pool.tile([P, QW], dt.bfloat16, tag="gxsb", bufs=2)
                    nc.scalar.activation(gxsb[:, :], pgxs[:, :], Act.Copy)
                    gysb = pool.tile([P, QW], dt.bfloat16, tag="gysb", bufs=2)
                    nc.scalar.activation(gysb[:, :], pgys[:, :], Act.Copy)
                    sxy = pool.tile([P, QW], dt.bfloat16, tag="sxy", bufs=2)
                    nc.vector.tensor_tensor(sxy[:, :], gxsb[:, :], gysb[:, :],
                                            Alu.mult)
                    nc.vector.tensor_scalar(csM[:, qs], sxy[:, :], 0.0, None,
                                            Alu.is_gt)
                    gx2 = pool.tile([P, QW], dt.bfloat16, tag="gx2", bufs=2)
                    nc.vector.tensor_tensor(gx2[:, :], gxsb[:, :], gxsb[:, :],
                                            Alu.mult)
                    gy2 = pool.tile([P, QW], dt.bfloat16, tag="gy2", bufs=2)
                    nc.vector.tensor_tensor(gy2[:, :], gysb[:, :], gysb[:, :],
                                            Alu.mult)
                    nc.vector.scalar_tensor_tensor(c2M[:, qs], gx2[:, :], TAN_HI2,
                                                   gy2[:, :], Alu.mult, Alu.is_lt)
                    nc.vector.scalar_tensor_tensor(c0M[:, qs], gx2[:, :], TAN_LO2,
                                                   gy2[:, :], Alu.mult, Alu.is_gt)

                # ---- stage C: shifts then NMS select + band (per quarter) ----
                ob = pool.tile([P, W], dt.uint8, tag="ob", bufs=1)
                for q in range(NQ):
                    pj = 1 + q * QW
                    # vertical shifts via SU/SD band matmuls (grad complete now)
                    pU = psq()
                    for ch in range(2):
                        nc.tensor.matmul(out=pU[:, ch * 512:(ch + 1) * 512],
                                         lhsT=wb_sb[:, go + 4 * P:go + 5 * P],
                                         rhs=grad[:, pj + ch * 512:
                                                  pj + ch * 512 + 512],
                                         start=True, stop=True)
                    nc.scalar.activation(gU[:, pj:pj + QW], pU[:, :], Act.Copy)
                    pD = psq()
                    for ch in range(2):
                        nc.tensor.matmul(out=pD[:, ch * 512:(ch + 1) * 512],
                                         lhsT=wb_sb[:, go + 5 * P:go + 6 * P],
                                         rhs=grad[:, pj + ch * 512:
                                                  pj + ch * 512 + 512],
                                         start=True, stop=True)
                    nc.scalar.activation(gD[:, pj:pj + QW], pD[:, :], Act.Copy)
                for q in range(NQ):
                    pj = 1 + q * QW
                    qs = slice(q * QW, (q + 1) * QW)
                    m1 = pool.tile([P, QW], dt.bfloat16, tag="m1", bufs=1)
                    nc.vector.tensor_tensor(m1[:, :], gD[:, pj + 1:pj + 1 + QW],
                                            gU[:, pj - 1:pj - 1 + QW], Alu.max)
                    msel = pool.tile([P, QW], dt.bfloat16, tag="msel", bufs=1)
                    nc.vector.tensor_tensor(msel[:, :], gD[:, pj - 1:pj - 1 + QW],
                                            gU[:, pj + 1:pj + 1 + QW], Alu.max)
                    m0 = pool.tile([P, QW], dt.bfloat16, tag="m0", bufs=1)
                    nc.vector.tensor_tensor(m0[:, :], grad[:, pj - 1:pj - 1 + QW],
                                            grad[:, pj + 1:pj + 1 + QW], Alu.max)
                    m2u = pool.tile([P, QW], dt.bfloat16, tag="m2u", bufs=1)
                    nc.vector.tensor_tensor(m2u[:, :], gU[:, pj:pj + QW],
                                            gD[:, pj:pj + QW], Alu.max)
                    nc.vector.copy_predicated(msel[:, :], csM[:, qs], m1[:, :])
                    nc.vector.copy_predicated(msel[:, :], c0M[:, qs], m0[:, :])
                    nc.vector.copy_predicated(msel[:, :], c2M[:, qs], m2u[:, :])
                    ig = pool.tile([P, QW], dt.bfloat16, tag="ig", bufs=1)
                    nc.vector.scalar_tensor_tensor(ig[:, :], msel[:, :], LOWER_T,
                                                   grad[:, pj:pj + QW],
                                                   Alu.max, Alu.is_lt)
                    nc.vector.scalar_tensor_tensor(ob[:, qs], grad[:, pj:pj + QW],
                                                   UPPER_T, ig[:, :],
                                                   Alu.is_le, Alu.mult)
                nc.sync.dma_start(out_d[t * P:(t + 1) * P, :], ob[:, :])

    import bass_rust
    bass_rust.move_matmul_waits_to_ldweights(nc.m)
    bass_rust.generate_event_semaphores(nc)
    nc.finalize()
    return nc


def _shard_inputs(img, gauss):
    imgf = np.ascontiguousarray(img[0])  # [3, H, W] f32
    in_maps = []
    for k in range(NCORES):
        xk = np.zeros((NCH, SH, W + 4), dtype=BF16)
        lo = k * RPC - HALO
        hi = k * RPC + RPC + HALO
        s0, s1 = max(lo, 0), min(hi, H)
        xk[:, s0 - lo:s1 - lo, 2:W + 2] = imgf[:, s0:s1, :].astype(BF16)
        wbk = _weights(gauss, is_top=(k == 0), is_bot=(k == NCORES - 1))
        in_maps.append({"x": xk, "wb": wbk})
    return in_maps


def _assemble(results):
    full = np.zeros((H, W), dtype=np.float32)
    for k in range(NCORES):
        ok = np.asarray(results[k]["out"])
        r0 = k * RPC
        for t in range(NT - 1):
            full[r0 + 120 * t:r0 + 120 * t + 120] = ok[P * t + 4:P * t + 124]
        full[r0 + 480:r0 + 512] = ok[(NT - 1) * P + 92:(NT - 1) * P + 124]
    return full.reshape(1, 1, H, W)


def _run(img, gauss, trace=False):
    nc = _build_nc()
    in_maps = _shard_inputs(np.asarray(img, np.float32), np.asarray(gauss, np.float32))
    res = run_bass_kernel_spmd(nc, in_maps, core_ids=list(range(NCORES)), trace=trace)
    return _assemble(res.results), res.exec_time_ns


def kernel(img=None, gauss=None, sobel=None, dir_w=None, **_):
    out, _ns = _run(img, gauss)
    return out


# revision 19
# speedup vs baseline: 1.8959x; 1.0014x over previous
import sys

sys.path.insert(0, "/opt/trn_rl_repo")

import numpy as np
import ml_dtypes

from concourse import bass, mybir
from concourse.tile import TileContext
from concourse.bass_utils import run_bass_kernel_spmd

dt = mybir.dt
Alu = mybir.AluOpType
Act = mybir.ActivationFunctionType

H = 4096
W = 4096
NCORES = 8
RPC = H // NCORES            # 512 output rows per core
HALO = 4                     # blur(2) + sobel(1) + nms(1)
SH = RPC + 2 * HALO          # 520 input rows per core
BASES = (0, 120, 240, 360, 392)
NT = len(BASES)
NCH = 3
P = 128
QW = 1024                    # quarter width (2 psum banks)
NQ = W // QW
BF16 = ml_dtypes.bfloat16

TAN_LO2 = float(np.float32(np.tan(3.14159 / 8)) ** 2)
TAN_HI2 = float(np.float32(np.tan(3 * 3.14159 / 8)) ** 2)
LOWER_T = 6.0
UPPER_T = 50.0

# wb column layout: 5 blur bands then [V121, NV121, U, U2, SU, SD] x {mid, t0, t4}
GO_MID = 5 * P
GO_T0 = GO_MID + 6 * P
GO_T4 = GO_T0 + 6 * P
WBW = GO_T4 + 6 * P          # 2944


def _band(taps, r):
    L = np.zeros((P, P), np.float32)
    for i, tv in enumerate(taps):
        L += np.float32(tv) * np.eye(P, k=r - i, dtype=np.float32)
    return L


def _weights(gauss, is_top, is_bot):
    g = np.asarray(gauss, np.float32)
    bg = _band(g, 2)
    v121 = _band([1.0, 2.0, 1.0], 1)
    u = _band([1.0, 0.0, -1.0], 1)
    su = _band([1.0], 1)
    sd = _band([1.0], -1)

    def group(zero_row, zero_su, zero_sd):
        mats = [v121.copy(), -v121, u.copy(), 2.0 * u, su.copy(), sd.copy()]
        if zero_row is not None:
            for idx in (0, 1, 2, 3):
                mats[idx][zero_row, :] = 0.0
            if zero_su:
                mats[4][zero_row, :] = 0.0
            if zero_sd:
                mats[5][zero_row, :] = 0.0
        return mats

    cols = [bg * g[d] for d in range(5)]
    cols += group(None, False, False)                       # mid
    cols += group(3 if is_top else None, True, False)       # t0 variant
    cols += group(124 if is_bot else None, False, True)     # t4 variant
    wb = np.concatenate(cols, axis=1)
    assert wb.shape == (P, WBW)
    return wb.astype(BF16)


def _build_nc():
    nc = bass.Bass(trn_type="TRN2")
    x_d = nc.dram_tensor("x", (NCH, SH, W + 4), dt.bfloat16, kind="ExternalInput")
    wb_d = nc.dram_tensor("wb", (P, WBW), dt.bfloat16, kind="ExternalInput")
    out_d = nc.dram_tensor("out", (NT * P, W), dt.uint8, kind="ExternalOutput")

    with TileContext(nc) as tc:
        with tc.tile_pool(name="sb", bufs=2) as pool, \
             tc.tile_pool(name="ps", bufs=2, space="PSUM") as pp:
            wb_sb = pool.tile([P, WBW], dt.bfloat16, tag="wb", bufs=1)
            nc.sync.dma_start(wb_sb[:, :], wb_d[:, :])

            _pq_n = [0]

            def psq():
                _pq_n[0] += 1
                return pp.tile([P, QW], dt.float32, tag="pq", bufs=4,
                               name=f"pq{_pq_n[0]}")

            def issue_x(t):
                base = BASES[t]
                xs = []
                for c in range(NCH):
                    x_sb = pool.tile([P, W + 4], dt.bfloat16, tag="x", bufs=4,
                                     name=f"x{t}_{c}")
                    nc.sync.dma_start(x_sb[:, :], x_d[c, base:base + P, :])
                    xs.append(x_sb)
                return xs

            xs_cur = issue_x(0)
            for t in range(NT):
                base = BASES[t]
                go = GO_T0 if t == 0 else (GO_T4 if t == NT - 1 else GO_MID)

                # ---- stage A: fused separable 5x5 blur per channel ----
                blurs = []
                for c in range(NCH):
                    bl = pool.tile([P, W + 2], dt.bfloat16, tag=f"bl{c}", bufs=2)
                    nc.vector.memset(bl[:, 0:1], 0.0)
                    nc.vector.memset(bl[:, W + 1:W + 2], 0.0)
                    for q in range(NQ):
                        ps = psq()
                        for d in range(5):
                            for ch in range(2):
                                j0 = q * QW + ch * 512
                                nc.tensor.matmul(
                                    out=ps[:, ch * 512:(ch + 1) * 512],
                                    lhsT=wb_sb[:, d * P:(d + 1) * P],
                                    rhs=xs_cur[c][:, j0 + d:j0 + d + 512],
                                    start=(d == 0), stop=(d == 4),
                                )
                        nc.scalar.activation(bl[:, 1 + q * QW:1 + (q + 1) * QW],
                                             ps[:, :], Act.Copy)
                    blurs.append(bl)
                if t + 1 < NT:
                    xs_cur = issue_x(t + 1)

                # blS = bl0+bl1+bl2 over full buffers (even base -> 2x DVE)
                blS = pool.tile([P, W + 2], dt.bfloat16, tag="blS", bufs=1)
                nc.vector.tensor_tensor(blS[:, :], blurs[0][:, :],
                                        blurs[1][:, :], Alu.add)
                nc.vector.tensor_tensor(blS[:, :], blS[:, :],
                                        blurs[2][:, :], Alu.add)

                # ---- stage B: grad, masks, shifts (per quarter) ----
                grad = pool.tile([P, W + 2], dt.bfloat16, tag="grad", bufs=2)
                gU = pool.tile([P, W + 2], dt.bfloat16, tag="gU", bufs=1)
                gD = pool.tile([P, W + 2], dt.bfloat16, tag="gD", bufs=1)
                for bufv in (grad, gU, gD):
                    nc.vector.memset(bufv[:, 0:1], 0.0)
                    nc.vector.memset(bufv[:, W + 1:W + 2], 0.0)
                csM = pool.tile([P, W], dt.uint8, tag="csM", bufs=2)
                c0M = pool.tile([P, W], dt.uint8, tag="c0M", bufs=2)
                c2M = pool.tile([P, W], dt.uint8, tag="c2M", bufs=2)

                def sobel_mm(src, q):
                    pj = 1 + q * QW
                    pgx = psq()
                    pgy = psq()
                    for i, (col, off) in enumerate(((go, -1), (go + P, 1))):
                        for ch in range(2):
                            nc.tensor.matmul(
                                out=pgx[:, ch * 512:(ch + 1) * 512],
                                lhsT=wb_sb[:, col:col + P],
                                rhs=src[:, pj + off + ch * 512:
                                        pj + off + ch * 512 + 512],
                                start=(i == 0), stop=(i == 1),
                            )
                    for i, (col, off) in enumerate(
                            ((go + 2 * P, -1), (go + 3 * P, 0), (go + 2 * P, 1))):
                        for ch in range(2):
                            nc.tensor.matmul(
                                out=pgy[:, ch * 512:(ch + 1) * 512],
                                lhsT=wb_sb[:, col:col + P],
                                rhs=src[:, pj + off + ch * 512:
                                        pj + off + ch * 512 + 512],
                                start=(i == 0), stop=(i == 2),
                            )
                    return pgx, pgy

                for q in range(NQ):
                    pj = 1 + q * QW
                    qs = slice(q * QW, (q + 1) * QW)
                    for c in range(NCH):
                        pgx, pgy = sobel_mm(blurs[c], q)
                        sqx = pool.tile([P, QW], dt.bfloat16, tag="sqx", bufs=2)
                        nc.scalar.activation(sqx[:, :], pgx[:, :], Act.Square)
                        sqy = pool.tile([P, QW], dt.bfloat16, tag="sqy", bufs=2)
                        nc.scalar.activation(sqy[:, :], pgy[:, :], Act.Square)
                        ss = pool.tile([P, QW], dt.bfloat16, tag="ss", bufs=2)
                        nc.vector.tensor_tensor(ss[:, :], sqx[:, :], sqy[:, :],
                                                Alu.add)
                        gslice = grad[:, pj:pj + QW]
                        if c == 0:
                            nc.scalar.activation(gslice, ss[:, :], Act.Sqrt)
                        else:
                            mg = pool.tile([P, QW], dt.bfloat16, tag="mg", bufs=2)
                            nc.scalar.activation(mg[:, :], ss[:, :], Act.Sqrt)
                            nc.vector.tensor_tensor(gslice, gslice, mg[:, :],
                                                    Alu.add)
                    # orientation masks from summed-blur sobel
                    pgxs, pgys = sobel_mm(blS, q)
                    gxsb = pool.tile([P, QW], dt.bfloat16, tag="gxsb", bufs=2)
                    nc.scalar.activation(gxsb[:, :], pgxs[:, :], Act.Copy)
                    gysb = pool.tile([P, QW], dt.bfloat16, tag="gysb", bufs=2)
                    nc.scalar.activation(gysb[:, :], pgys[:, :], Act.Copy)
                    sxy = pool.tile([P, QW], dt.bfloat16, tag="sxy", bufs=2)
                    nc.vector.tensor_tensor(sxy[:, :], gxsb[:, :], gysb[:, :],
                                            Alu.mult)
                    nc.vector.tensor_scalar(csM[:, qs], sxy[:, :], 0.0, None,
                                            Alu.is_gt)
                    gx2 = pool.tile([P, QW], dt.bfloat16, tag="gx2", bufs=2)
                    nc.vector.tensor_tensor(gx2[:, :], gxsb[:, :], gxsb[:, :],
                                            Alu.mult)
                    gy2 = pool.tile([P, QW], dt.bfloat16, tag="gy2", bufs=2)
                    nc.vector.tensor_tensor(gy2[:, :], gysb[:, :], gysb[:, :],
                                            Alu.mult)
                    nc.vector.scalar_tensor_tensor(c2M[:, qs], gx2[:, :], TAN_HI2,
                                                   gy2[:, :], Alu.mult, Alu.is_lt)
                    nc.vector.scalar_tensor_tensor(c0M[:, qs], gx2[:, :], TAN_LO2,
                                                   gy2[:, :], Alu.mult, Alu.is_gt)

                # ---- stage C: shifts then NMS select + band (per quarter) ----
                ob = pool.tile([P, W], dt.uint8, tag="ob", bufs=1)
                for q in range(NQ):
                    pj = 1 + q * QW
                    # vertical shifts via SU/SD band matmuls (grad complete now)
                    pU = psq()
                    for ch in range(2):
                        nc.tensor.matmul(out=pU[:, ch * 512:(ch + 1) * 512],
                                         lhsT=wb_sb[:, go + 4 * P:go + 5 * P],
                                         rhs=grad[:, pj + ch * 512:
                                                  pj + ch * 512 + 512],
                                         start=True, stop=True)
                    nc.scalar.activation(gU[:, pj:pj + QW], pU[:, :], Act.Copy)
                    pD = psq()
                    for ch in range(2):
                        nc.tensor.matmul(out=pD[:, ch * 512:(ch + 1) * 512],
                                         lhsT=wb_sb[:, go + 5 * P:go + 6 * P],
                                         rhs=grad[:, pj + ch * 512:
                                                  pj + ch * 512 + 512],
                                         start=True, stop=True)
                    nc.scalar.activation(gD[:, pj:pj + QW], pD[:, :], Act.Copy)
                for q in range(NQ):
                    pj = 1 + q * QW
                    qs = slice(q * QW, (q + 1) * QW)
                    m1 = pool.tile([P, QW], dt.bfloat16, tag="m1", bufs=1)
                    nc.vector.tensor_tensor(m1[:, :], gD[:, pj + 1:pj + 1 + QW],
                                            gU[:, pj - 1:pj - 1 + QW], Alu.max)
                    msel = pool.tile([P, QW], dt.bfloat16, tag="msel", bufs=1)
                    nc.vector.tensor_tensor(msel[:, :], gD[:, pj - 1:pj - 1 + QW],
                                            gU[:, pj + 1:pj + 1 + QW], Alu.max)
                    m0 = pool.tile([P, QW], dt.bfloat16, tag="m0", bufs=1)
                    nc.vector.tensor_tensor(m0[:, :], grad[:, pj - 1:pj - 1 + QW],
                                            grad[:, pj + 1:pj + 1 + QW], Alu.max)
                    m2u = pool.tile([P, QW], dt.bfloat16, tag="m2u", bufs=1)
                    nc.vector.tensor_tensor(m2u[:, :], gU[:, pj:pj + QW],
                                            gD[:, pj:pj + QW], Alu.max)
                    nc.vector.copy_predicated(msel[:, :], csM[:, qs], m1[:, :])
                    nc.vector.copy_predicated(msel[:, :], c0M[:, qs], m0[:, :])
                    nc.vector.copy_predicated(msel[:, :], c2M[:, qs], m2u[:, :])
                    ig = pool.tile([P, QW], dt.bfloat16, tag="ig", bufs=1)
                    nc.vector.scalar_tensor_tensor(ig[:, :], msel[:, :], LOWER_T,
                                                   grad[:, pj:pj + QW],
                                                   Alu.max, Alu.is_lt)
                    nc.vector.scalar_tensor_tensor(ob[:, qs], grad[:, pj:pj + QW],
                                                   UPPER_T, ig[:, :],
                                                   Alu.is_le, Alu.mult)
                nc.sync.dma_start(out_d[t * P:(t + 1) * P, :], ob[:, :])

    import bass_rust
    bass_rust.move_matmul_waits_to_ldweights(nc.m)
    bass_rust.generate_event_semaphores(nc)
    nc.finalize()
    return nc


def _shard_inputs(img, gauss):
    imgf = np.ascontiguousarray(img[0])  # [3, H, W] f32
    in_maps = []
    for k in range(NCORES):
        xk = np.zeros((NCH, SH, W + 4), dtype=BF16)
        lo = k * RPC - HALO
        hi = k * RPC + RPC + HALO
        s0, s1 = max(lo, 0), min(hi, H)
        xk[:, s0 - lo:s1 - lo, 2:W + 2] = imgf[:, s0:s1, :].astype(BF16)
        wbk = _weights(gauss, is_top=(k == 0), is_bot=(k == NCORES - 1))
        in_maps.append({"x": xk, "wb": wbk})
    return in_maps


def _assemble(results):
    full = np.zeros((H, W), dtype=np.float32)
    for k in range(NCORES):
        ok = np.asarray(results[k]["out"])
        r0 = k * RPC
        for t in range(NT - 1):
            full[r0 + 120 * t:r0 + 120 * t + 120] = ok[P * t + 4:P * t + 124]
        full[r0 + 480:r0 + 512] = ok[(NT - 1) * P + 92:(NT - 1) * P + 124]
    return full.reshape(1, 1, H, W)


def _run(img, gauss, trace=False):
    nc = _build_nc()
    in_maps = _shard_inputs(np.asarray(img, np.float32), np.asarray(gauss, np.float32))
    res = run_bass_kernel_spmd(nc, in_maps, core_ids=list(range(NCORES)), trace=trace)
    return _assemble(res.results), res.exec_time_ns


def kernel(img=None, gauss=None, sobel=None, dir_w=None, **_):
    out, _ns = _run(img, gauss)
    return out
